# revision 38
# baseline (speedup 1.0000x reference)
"""VQ-VAE forward (quant conv -> codebook argmin -> losses -> post-quant conv)
as a data-parallel 8-core Trainium2 Bass kernel.

Sharding: data-parallel over batch B=32 -> 4 images per core. Codebook and
1x1-conv weights are folded host-side and replicated. The scalar loss is
assembled on the host from per-core partial sums (the all-reduce of the
sharding hint, done at gather time).

Math notes (exact reformulations of the reference):
  - argmin_k |x_t - c_k|^2 == argmax_k s[t,k],
        s[t,k] = z_t . (quant_w^T c_k) + (quant_b . c_k - 0.5|c_k|^2)
    so the quant conv folds into the codebook and a per-code bias row
    (ones-row augmentation -> K=33 matmul straight from z).
  - scores are computed in two-term fp16 splits (z = z0+z1, cbf = c0+c1,
    s ~= z0c0 + z0c1 + z1c0 accumulated in f32 PSUM). fp16 has an 11-bit
    mantissa so the dropped z1c1 term is ~2^-24 relative: the argmax
    matches a pure-f32 computation except for ~1e-5-probability near-ties.
    This runs the TensorE at 1 cycle/row instead of fp32's 4.
  - argmax extraction: two-axis trick. gm1[t,g] = max_j s[t,16g+j],
    gm2[t,j] = max_g s[t,16g+j]; with m = max s: g* = argmax gm1,
    j* = argmax gm2, idx = 16 g* + j*.
  - q_st == q numerically, so out = W2[idx] with
    W2 = codebook @ post_quant_w^T + post_quant_b (dma_gather table).
  - commit and codebook losses are equal in value:
        loss = 2*(sum|x_t|^2 - 2*sum_t s_max(t)) / (N*D)

Device pipeline (per core, 16384 tokens in 128 token-tiles):
  - score matmuls 2-way row-group packed (PE rows 0-32 / 64-96; z and the
    folded codebook replicated at partition 64).
  - per tile: two DVE reduces (gm1, gm2) straight from PSUM.
  - tiles processed in 4 quarters; each quarter runs batched extraction,
    then 4 dma_gathers (1024 idx each - SWDGE ring is 128 descriptors
    deep) and its output DMA, overlapping later quarters' compute.
"""

import functools
import sys

import numpy as np

sys.path.insert(0, "/opt/trn_rl_repo")

from concourse import bacc, bass, bass_utils, mybir, tile  # noqa: E402

F32 = mybir.dt.float32
F16 = mybir.dt.float16
I16 = mybir.dt.int16
AF = mybir.ActivationFunctionType
ALU = mybir.AluOpType
AX = mybir.AxisListType

# Problem geometry (hardcoded per spec).
B, C, H, W = 32, 32, 64, 64
D, K = 32, 512
NCORES = 8
BPC = B // NCORES            # images per core
TOK = BPC * H * W            # tokens per core = 16384
NTILE = TOK // 128           # 128 token tiles of 128 tokens
G, J = 32, 16                # codebook split: 32 groups of 16
PH = [32, 32, 32, 16, 16]    # pipeline phase sizes (tiles, multiples of 8)
PS = [0, 32, 64, 96, 112, 128]  # phase tile starts (cumsum)


def _build_graph():
    nc = bacc.Bacc("TRN2", target_bir_lowering=False, debug=False,
                   num_devices=NCORES)

    # ---- DRAM I/O ----
    z0d = nc.dram_tensor("z0", [34, TOK], F16, kind="ExternalInput").ap()
    z1d = nc.dram_tensor("z1", [34, TOK], F16, kind="ExternalInput").ap()
    c0d = nc.dram_tensor("c0", [33, K], F16, kind="ExternalInput").ap()
    c1d = nc.dram_tensor("c1", [33, K], F16, kind="ExternalInput").ap()
    qwT = nc.dram_tensor("qwT", [C, D], F16, kind="ExternalInput").ap()
    qb = nc.dram_tensor("qb", [D, 1], F32, kind="ExternalInput").ap()
    w2p = nc.dram_tensor("w2p", [K, 64], F32, kind="ExternalInput").ap()
    iota32 = nc.dram_tensor("iota32", [128, G], F32, kind="ExternalInput").ap()
    iota16 = nc.dram_tensor("iota16", [128, J], F32, kind="ExternalInput").ap()

    qg = nc.dram_tensor("qg", [128, NTILE, 64], F32, kind="ExternalOutput").ap()
    idxm = nc.dram_tensor("idxm", [128, NTILE], F32, kind="ExternalOutput").ap()
    mrow = nc.dram_tensor("mrow", [128, NTILE], F32, kind="ExternalOutput").ap()
    ssqb = nc.dram_tensor("ssqb", [D, TOK // 512], F32, kind="ExternalOutput").ap()
    idxscr = nc.dram_tensor("idxscr", [128, NTILE], I16, kind="Internal").ap()

    with tile.TileContext(nc) as tc:
        with tc.tile_pool(name="big", bufs=1) as big, \
             tc.tile_pool(name="pp", bufs=4, space="PSUM") as pp:

            # Persistent SBUF tensors.  z + codebook live twice (partition 0
            # and 64) so score matmuls run in two concurrent PE row groups.
            z0_sb = big.tile([128, TOK], F16)    # rows 0-33 & 64-97 used
            z1_sb = big.tile([128, TOK], F16)
            c0_sb = big.tile([128, K], F16)      # rows 0-32 & 64-96 used
            c1_sb = big.tile([128, K], F16)
            qwT_sb = big.tile([C, D], F16)
            qb_sb = big.tile([D, 1], F32)
            io32_sb = big.tile([128, G], F32)
            io16_sb = big.tile([128, J], F32)
            gm1b = big.tile([128, NTILE, G], F32)
            gm2b = big.tile([128, NTILE, J], F32)
            ghb = big.tile([128, NTILE, G], F32)
            gh2b = big.tile([128, NTILE, J], F32)
            mb = big.tile([128, NTILE], F32)
            gstar = big.tile([128, NTILE], F32)
            jstar = big.tile([128, NTILE], F32)
            idxf = big.tile([128, NTILE], F32)
            idxi = big.tile([128, NTILE], I16)
            idxr = big.tile([128, TOK // 16], I16)
            qgs = big.tile([128, NTILE, 64], F32)
            scr = big.tile([D, 512], F32)        # ACT main-out scratch
            ssq_sb = big.tile([D, TOK // 512], F32)

            # ---- Load constants + z (chunked for overlap) ----
            from concourse import library_config
            nc.gpsimd.load_library(library_config.mlp)
            nc.vector.memset(idxr[:, 0:64], 0)
            nc.gpsimd.dma_gather(
                out_ap=qgs[:, 0:8, :], in_ap=w2p[:],
                idxs_ap=idxr[:, 0:64], num_idxs=1024,
                num_idxs_reg=1024, elem_size=64)
            for t_sb, t_d in ((c0_sb, c0d), (c1_sb, c1d)):
                nc.sync.dma_start(t_sb[0:33, :], t_d[:])
                nc.sync.dma_start(t_sb[64:97, :], t_d[:])
            nc.sync.dma_start(qwT_sb[:], qwT[:])
            nc.sync.dma_start(qb_sb[:], qb[:])
            nc.sync.dma_start(io32_sb[:], iota32[:])
            nc.sync.dma_start(io16_sb[:], iota16[:])
            for b in range(2 * BPC):
                sl = slice(b * 2048, (b + 1) * 2048)
                for t_sb, t_d in ((z0_sb, z0d), (z1_sb, z1d)):
                    nc.sync.dma_start(t_sb[0:34, sl], t_d[:, sl])
                    nc.sync.dma_start(t_sb[64:98, sl], t_d[:, sl])

            def do_pair(pair):
                t0, t1 = 2 * pair, 2 * pair + 1
                sp = pp.tile([128, 2, K], F32, tag="sp")
                for i, t, base in ((0, t0, 0), (1, t1, 64)):
                    ts = slice(t * 128, (t + 1) * 128)
                    bs = slice(base, base + 33)
                    tp = (base, 0)
                    nc.tensor.matmul(sp[:, i, :], lhsT=z0_sb[bs, ts],
                                     rhs=c0_sb[bs, :], start=True, stop=False,
                                     tile_position=tp)
                    nc.tensor.matmul(sp[:, i, :], lhsT=z0_sb[bs, ts],
                                     rhs=c1_sb[bs, :], start=False, stop=False,
                                     tile_position=tp)
                    nc.tensor.matmul(sp[:, i, :], lhsT=z1_sb[bs, ts],
                                     rhs=c0_sb[bs, :], start=False, stop=True,
                                     tile_position=tp)
                nc.vector.tensor_reduce(
                    gm1b[:, t0:t0 + 2, :],
                    sp[:].rearrange("p two (g j) -> p two g j", j=J),
                    axis=AX.X, op=ALU.max)
                nc.vector.tensor_reduce(
                    gm2b[:, t0:t0 + 2, :],
                    sp[:].rearrange("p two (g j) -> p two j g", j=J),
                    axis=AX.X, op=ALU.max)

            def do_ssq(c):
                # x ~= qw z0; ssq += sum (x + qb)^2.  Dropping the z1 term
                # perturbs the scalar loss by ~2e-5 relative - negligible.
                cs = slice(c * 512, (c + 1) * 512)
                xt = pp.tile([128, 2, K], F32, tag="sp")
                xp = xt[0:D, 0, :]
                nc.tensor.matmul(xp, lhsT=qwT_sb[:], rhs=z0_sb[0:32, cs],
                                 start=True, stop=True)
                nc.scalar.activation(scr[:], xp, AF.Square, bias=qb_sb[:],
                                     accum_out=ssq_sb[:, c:c + 1])

            def extract_quarter(q):
                h = slice(PS[q], PS[q + 1])
                QT = PH[q]
                nc.vector.tensor_reduce(mb[:, h], gm1b[:, h, :], axis=AX.X,
                                        op=ALU.max)
                mexp = mb[:, h].unsqueeze(2)
                nc.vector.tensor_tensor(ghb[:, h, :], gm1b[:, h, :],
                                        mexp.broadcast_to([128, QT, G]),
                                        op=ALU.is_equal)
                nc.vector.tensor_tensor(gh2b[:, h, :], gm2b[:, h, :],
                                        mexp.broadcast_to([128, QT, J]),
                                        op=ALU.is_equal)
                nc.vector.tensor_tensor(
                    ghb[:, h, :], ghb[:, h, :],
                    io32_sb[:].unsqueeze(1).broadcast_to([128, QT, G]),
                    op=ALU.mult)
                nc.vector.tensor_tensor(
                    gh2b[:, h, :], gh2b[:, h, :],
                    io16_sb[:].unsqueeze(1).broadcast_to([128, QT, J]),
                    op=ALU.mult)
                nc.vector.tensor_reduce(gstar[:, h], ghb[:, h, :], axis=AX.X,
                                        op=ALU.add)
                nc.vector.tensor_reduce(jstar[:, h], gh2b[:, h, :], axis=AX.X,
                                        op=ALU.add)
                nc.vector.tensor_scalar(idxf[:, h], gstar[:, h], float(J),
                                        None, op0=ALU.mult)
                nc.vector.tensor_tensor(idxf[:, h], idxf[:, h], jstar[:, h],
                                        op=ALU.add)
                nc.vector.tensor_copy(idxi[:, h], idxf[:, h])

            def gather_quarter(q):
                # Wrapped idx layout, phase-local s8-major:
                #   idxs[p, 8*PS[q] + s8*PH[q] + tl] = idxi[p + 16*s8, PS[q]+tl]
                # => gather slot i = s*16+p holds token
                #   t = (PS[q] + tl)*128 + 16*s8 + p.
                h = slice(PS[q], PS[q + 1])
                nc.scalar.dma_start(idxscr[:, h], idxi[:, h])
                src = idxscr.rearrange("(s8 p) tile -> p s8 tile", p=16)[
                    :, :, h]
                for grp in range(8):
                    dst = idxr[16 * grp:16 * (grp + 1),
                               8 * PS[q]:8 * PS[q + 1]].rearrange(
                        "p (s8 tl) -> p s8 tl", s8=8)
                    nc.scalar.dma_start(dst, src)
                # PH[q]/8 gathers of 1024 idx; gather c covers idxr columns
                # [8 PS[q] + 64c, +64) -> qgs rows [PS[q] + 8c, +8).
                for c in range(PH[q] // 8):
                    s_lo = 8 * PS[q] + 64 * c
                    nc.gpsimd.dma_gather(
                        out_ap=qgs[:, s_lo // 8:s_lo // 8 + 8, :],
                        in_ap=w2p[:],
                        idxs_ap=idxr[:, s_lo:s_lo + 64],
                        num_idxs=1024,
                        num_idxs_reg=1024,
                        elem_size=64,
                    )
                nc.sync.dma_start(qg[:, h, :], qgs[:, h, :])

            # ---- Main schedule ----
            for pair in range(NTILE // 2):
                do_pair(pair)
                for q in range(len(PH) - 1):
                    if pair == PS[q + 1] // 2 - 1:
                        extract_quarter(q)
                        gather_quarter(q)
            extract_quarter(len(PH) - 1)
            gather_quarter(len(PH) - 1)
            for c in range(TOK // 512):
                do_ssq(c)
            nc.sync.dma_start(ssqb[:], ssq_sb[:])
            nc.sync.dma_start(mrow[:], mb[:])
            nc.sync.dma_start(idxm[:], idxf[:])

    nc.compile()
    return nc


@functools.lru_cache(maxsize=1)
def _graph():
    return _build_graph()


def _split16(a):
    hi = a.astype(np.float16)
    lo = (a - hi.astype(np.float32)).astype(np.float16)
    return hi, lo


def kernel(z, quant_w, quant_b, codebook, post_quant_w, post_quant_b):
    z = np.asarray(z, dtype=np.float32)
    quant_w = np.asarray(quant_w, dtype=np.float32)
    quant_b = np.asarray(quant_b, dtype=np.float32)
    codebook = np.asarray(codebook, dtype=np.float32)
    post_quant_w = np.asarray(post_quant_w, dtype=np.float32)
    post_quant_b = np.asarray(post_quant_b, dtype=np.float32)

    # ---- Host-side weight folding (tiny, O(K*D)) ----
    cbf = (codebook @ quant_w).astype(np.float32)              # [K, C]
    bias = (codebook @ quant_b
            - 0.5 * np.sum(codebook * codebook, axis=1)).astype(np.float32)
    c0b, c1b = _split16(cbf.T)                                 # [C, K]
    b0, b1 = _split16(bias[None, :])
    c0 = np.concatenate([c0b, b0], axis=0)                     # [33, K] fp16
    c1 = np.concatenate([c1b, b1], axis=0)
    w2 = (codebook @ post_quant_w.T + post_quant_b[None, :]).astype(np.float32)
    w2p = np.zeros((K, 64), np.float32)
    w2p[:, 0:D] = w2
    qwT = quant_w.T.astype(np.float16)                         # [C, D]
    qb = quant_b.reshape(D, 1).astype(np.float32)
    io32 = np.broadcast_to(np.arange(G, dtype=np.float32), (128, G)).copy()
    io16 = np.broadcast_to(np.arange(J, dtype=np.float32), (128, J)).copy()

    in_maps = []
    for core in range(NCORES):
        zc = z[BPC * core:BPC * (core + 1)]                    # [4, C, H, W]
        zc = zc.transpose(1, 0, 2, 3).reshape(C, TOK)          # [C, 16384]
        zh, zl = _split16(zc)
        z0 = np.zeros((34, TOK), np.float16)
        z0[0:C] = zh
        z0[C] = 1.0
        z1 = np.zeros((34, TOK), np.float16)
        z1[0:C] = zl
        in_maps.append({
            "z0": z0, "z1": z1, "c0": c0, "c1": c1, "qwT": qwT, "qb": qb,
            "w2p": w2p, "iota32": io32, "iota16": io16,
        })

    nc = _graph()
    res = bass_utils.run_bass_kernel_spmd(nc, in_maps,
                                          core_ids=list(range(NCORES)))
    outs = res.results

    out = np.empty((B, C, H, W), np.float32)
    idx = np.empty((B, H * W), np.int32)
    ssq_total = 0.0
    smax_total = 0.0
    for core in range(NCORES):
        r = outs[core]
        # qg[16*(tl%8)+p, PS[q] + s8*(PH[q]//8) + tl//8, :] = token
        #   (PS[q] + tl)*128 + 16*s8 + p
        o_tok = np.empty((TOK, 64), np.float32)
        for ph in range(len(PH)):
            n8 = PH[ph] // 8
            qv = r["qg"][:, PS[ph]:PS[ph + 1], :].reshape(8, 16, 8, n8, 64)
            o_tok[PS[ph] * 128:PS[ph + 1] * 128] = np.transpose(
                qv, (3, 0, 2, 1, 4)).reshape(PH[ph] * 128, 64)
        o_tok = o_tok[:, 0:D]
        o_img = o_tok.reshape(BPC, H * W, D).transpose(0, 2, 1)
        out[BPC * core:BPC * (core + 1)] = o_img.reshape(BPC, D, H, W)
        idxc = r["idxm"].T.reshape(TOK).astype(np.int32)
        idx[BPC * core:BPC * (core + 1)] = idxc.reshape(BPC, H * W)
        ssq_total += float(r["ssqb"].sum())
        smax_total += float(r["mrow"].sum())

    n_tok = B * H * W
    loss = np.float32(2.0 * (ssq_total - 2.0 * smax_total) / (n_tok * D))
    return out, idx, loss


# revision 39
# speedup vs baseline: 1.0233x; 1.0233x over previous
"""VQ-VAE forward (quant conv -> codebook argmin -> losses -> post-quant conv)
as a data-parallel 8-core Trainium2 Bass kernel.

Sharding: data-parallel over batch B=32 -> 4 images per core. Codebook and
1x1-conv weights are folded host-side and replicated. The scalar loss is
assembled on the host from per-core partial sums (the all-reduce of the
sharding hint, done at gather time).

Math notes (exact reformulations of the reference):
  - argmin_k |x_t - c_k|^2 == argmax_k s[t,k],
        s[t,k] = z_t . (quant_w^T c_k) + (quant_b . c_k - 0.5|c_k|^2)
    so the quant conv folds into the codebook and a per-code bias row
    (ones-row augmentation -> K=33 matmul straight from z).
  - scores are computed in two-term fp16 splits (z = z0+z1, cbf = c0+c1,
    s ~= z0c0 + z0c1 + z1c0 accumulated in f32 PSUM). fp16 has an 11-bit
    mantissa so the dropped z1c1 term is ~2^-24 relative: the argmax
    matches a pure-f32 computation except for ~1e-5-probability near-ties.
    This runs the TensorE at 1 cycle/row instead of fp32's 4.
  - argmax extraction: two-axis trick. gm1[t,g] = max_j s[t,16g+j],
    gm2[t,j] = max_g s[t,16g+j]; with m = max s: g* = argmax gm1,
    j* = argmax gm2, idx = 16 g* + j*.
  - q_st == q numerically, so out = W2[idx] with
    W2 = codebook @ post_quant_w^T + post_quant_b (dma_gather table).
  - commit and codebook losses are equal in value:
        loss = 2*(sum|x_t|^2 - 2*sum_t s_max(t)) / (N*D)

Device pipeline (per core, 16384 tokens in 128 token-tiles):
  - score matmuls 2-way row-group packed (PE rows 0-32 / 64-96; z and the
    folded codebook replicated at partition 64).
  - per tile: two DVE reduces (gm1, gm2) straight from PSUM.
  - tiles processed in 4 quarters; each quarter runs batched extraction,
    then 4 dma_gathers (1024 idx each - SWDGE ring is 128 descriptors
    deep) and its output DMA, overlapping later quarters' compute.
"""

import functools
import sys

import numpy as np

sys.path.insert(0, "/opt/trn_rl_repo")

from concourse import bacc, bass, bass_utils, mybir, tile  # noqa: E402

F32 = mybir.dt.float32
F16 = mybir.dt.float16
I16 = mybir.dt.int16
AF = mybir.ActivationFunctionType
ALU = mybir.AluOpType
AX = mybir.AxisListType

# Problem geometry (hardcoded per spec).
B, C, H, W = 32, 32, 64, 64
D, K = 32, 512
NCORES = 8
BPC = B // NCORES            # images per core
TOK = BPC * H * W            # tokens per core = 16384
NTILE = TOK // 128           # 128 token tiles of 128 tokens
G, J = 32, 16                # codebook split: 32 groups of 16
PH = [32, 32, 32, 32]        # pipeline phase sizes (tiles, multiples of 8)
PS = [0, 32, 64, 96, 128]    # phase tile starts (cumsum)


def _build_graph():
    nc = bacc.Bacc("TRN2", target_bir_lowering=False, debug=False,
                   num_devices=NCORES)

    # ---- DRAM I/O ----
    z0d = nc.dram_tensor("z0", [34, TOK], F16, kind="ExternalInput").ap()
    z1d = nc.dram_tensor("z1", [34, TOK], F16, kind="ExternalInput").ap()
    c0d = nc.dram_tensor("c0", [33, K], F16, kind="ExternalInput").ap()
    c1d = nc.dram_tensor("c1", [33, K], F16, kind="ExternalInput").ap()
    qwT = nc.dram_tensor("qwT", [C, D], F16, kind="ExternalInput").ap()
    qb = nc.dram_tensor("qb", [D, 1], F32, kind="ExternalInput").ap()
    w2p = nc.dram_tensor("w2p", [K, 64], F32, kind="ExternalInput").ap()
    iota32 = nc.dram_tensor("iota32", [128, G], F32, kind="ExternalInput").ap()
    iota16 = nc.dram_tensor("iota16", [128, J], F32, kind="ExternalInput").ap()

    qg = nc.dram_tensor("qg", [128, NTILE, 64], F32, kind="ExternalOutput").ap()
    idxm = nc.dram_tensor("idxm", [128, NTILE], F32, kind="ExternalOutput").ap()
    mrow = nc.dram_tensor("mrow", [128, NTILE], F32, kind="ExternalOutput").ap()
    ssqb = nc.dram_tensor("ssqb", [D, TOK // 512], F32, kind="ExternalOutput").ap()
    idxscr = nc.dram_tensor("idxscr", [128, NTILE], I16, kind="Internal").ap()

    with tile.TileContext(nc) as tc:
        with tc.tile_pool(name="big", bufs=1) as big, \
             tc.tile_pool(name="pp", bufs=3, space="PSUM") as pp, \
             tc.tile_pool(name="px", bufs=1, space="PSUM") as px:

            # Persistent SBUF tensors.  z + codebook live twice (partition 0
            # and 64) so score matmuls run in two concurrent PE row groups.
            z0_sb = big.tile([128, TOK], F16)    # rows 0-33 & 64-97 used
            z1_sb = big.tile([128, TOK], F16)
            c0_sb = big.tile([128, K], F16)      # rows 0-32 & 64-96 used
            c1_sb = big.tile([128, K], F16)
            qwT_sb = big.tile([C, D], F16)
            qb_sb = big.tile([D, 1], F32)
            io32_sb = big.tile([128, G], F32)
            io16_sb = big.tile([128, J], F32)
            gm1b = big.tile([128, NTILE, G], F32)
            gm2b = big.tile([128, NTILE, J], F32)
            ghb = big.tile([128, NTILE, G], F32)
            gh2b = big.tile([128, NTILE, J], F32)
            mb = big.tile([128, NTILE], F32)
            gstar = big.tile([128, NTILE], F32)
            jstar = big.tile([128, NTILE], F32)
            idxf = big.tile([128, NTILE], F32)
            idxi = big.tile([128, NTILE], I16)
            idxr = big.tile([128, TOK // 16], I16)
            qgs = big.tile([128, NTILE, 64], F32)
            scr = big.tile([D, 512], F32)        # ACT main-out scratch
            ssq_sb = big.tile([D, TOK // 512], F32)

            # ---- Load constants + z (chunked for overlap) ----
            from concourse import library_config
            nc.gpsimd.load_library(library_config.mlp)
            nc.vector.memset(idxr[:, 0:64], 0)
            nc.gpsimd.dma_gather(
                out_ap=qgs[:, 0:8, :], in_ap=w2p[:],
                idxs_ap=idxr[:, 0:64], num_idxs=1024,
                num_idxs_reg=1024, elem_size=64)
            for t_sb, t_d in ((c0_sb, c0d), (c1_sb, c1d)):
                nc.sync.dma_start(t_sb[0:33, :], t_d[:])
                nc.sync.dma_start(t_sb[64:97, :], t_d[:])
            nc.sync.dma_start(qwT_sb[:], qwT[:])
            nc.sync.dma_start(qb_sb[:], qb[:])
            nc.sync.dma_start(io32_sb[:], iota32[:])
            nc.sync.dma_start(io16_sb[:], iota16[:])
            for b in range(2 * BPC):
                sl = slice(b * 2048, (b + 1) * 2048)
                for t_sb, t_d in ((z0_sb, z0d), (z1_sb, z1d)):
                    nc.sync.dma_start(t_sb[0:34, sl], t_d[:, sl])
                    nc.sync.dma_start(t_sb[64:98, sl], t_d[:, sl])

            def do_pair(pair):
                t0, t1 = 2 * pair, 2 * pair + 1
                sp = pp.tile([128, 2, K], F32, tag="sp")
                for i, t, base in ((0, t0, 0), (1, t1, 64)):
                    ts = slice(t * 128, (t + 1) * 128)
                    bs = slice(base, base + 33)
                    tp = (base, 0)
                    nc.tensor.matmul(sp[:, i, :], lhsT=z0_sb[bs, ts],
                                     rhs=c0_sb[bs, :], start=True, stop=False,
                                     tile_position=tp)
                    nc.tensor.matmul(sp[:, i, :], lhsT=z0_sb[bs, ts],
                                     rhs=c1_sb[bs, :], start=False, stop=False,
                                     tile_position=tp)
                    nc.tensor.matmul(sp[:, i, :], lhsT=z1_sb[bs, ts],
                                     rhs=c0_sb[bs, :], start=False, stop=True,
                                     tile_position=tp)
                nc.vector.tensor_reduce(
                    gm1b[:, t0:t0 + 2, :],
                    sp[:].rearrange("p two (g j) -> p two g j", j=J),
                    axis=AX.X, op=ALU.max)
                nc.vector.tensor_reduce(
                    gm2b[:, t0:t0 + 2, :],
                    sp[:].rearrange("p two (g j) -> p two j g", j=J),
                    axis=AX.X, op=ALU.max)

            def do_ssq(c):
                # x ~= qw z0; ssq += sum (x + qb)^2.  Dropping the z1 term
                # perturbs the scalar loss by ~2e-5 relative - negligible.
                cs = slice(c * 512, (c + 1) * 512)
                xp = px.tile([D, 512], F32, tag="xp")
                nc.tensor.matmul(xp[:], lhsT=qwT_sb[:], rhs=z0_sb[0:32, cs],
                                 start=True, stop=True)
                nc.scalar.activation(scr[:], xp[:], AF.Square, bias=qb_sb[:],
                                     accum_out=ssq_sb[:, c:c + 1])

            def extract_quarter(q):
                h = slice(PS[q], PS[q + 1])
                QT = PH[q]
                nc.vector.tensor_reduce(mb[:, h], gm1b[:, h, :], axis=AX.X,
                                        op=ALU.max)
                mexp = mb[:, h].unsqueeze(2)
                nc.vector.tensor_tensor(ghb[:, h, :], gm1b[:, h, :],
                                        mexp.broadcast_to([128, QT, G]),
                                        op=ALU.is_equal)
                nc.vector.tensor_tensor(gh2b[:, h, :], gm2b[:, h, :],
                                        mexp.broadcast_to([128, QT, J]),
                                        op=ALU.is_equal)
                nc.vector.tensor_tensor(
                    ghb[:, h, :], ghb[:, h, :],
                    io32_sb[:].unsqueeze(1).broadcast_to([128, QT, G]),
                    op=ALU.mult)
                nc.vector.tensor_tensor(
                    gh2b[:, h, :], gh2b[:, h, :],
                    io16_sb[:].unsqueeze(1).broadcast_to([128, QT, J]),
                    op=ALU.mult)
                nc.vector.tensor_reduce(gstar[:, h], ghb[:, h, :], axis=AX.X,
                                        op=ALU.add)
                nc.vector.tensor_reduce(jstar[:, h], gh2b[:, h, :], axis=AX.X,
                                        op=ALU.add)
                nc.vector.tensor_scalar(idxf[:, h], gstar[:, h], float(J),
                                        None, op0=ALU.mult)
                nc.vector.tensor_tensor(idxf[:, h], idxf[:, h], jstar[:, h],
                                        op=ALU.add)
                nc.vector.tensor_copy(idxi[:, h], idxf[:, h])

            def gather_quarter(q):
                # Wrapped idx layout, phase-local s8-major:
                #   idxs[p, 8*PS[q] + s8*PH[q] + tl] = idxi[p + 16*s8, PS[q]+tl]
                # => gather slot i = s*16+p holds token
                #   t = (PS[q] + tl)*128 + 16*s8 + p.
                h = slice(PS[q], PS[q + 1])
                nc.scalar.dma_start(idxscr[:, h], idxi[:, h])
                src = idxscr.rearrange("(s8 p) tile -> p s8 tile", p=16)[
                    :, :, h]
                for grp in range(8):
                    dst = idxr[16 * grp:16 * (grp + 1),
                               8 * PS[q]:8 * PS[q + 1]].rearrange(
                        "p (s8 tl) -> p s8 tl", s8=8)
                    nc.scalar.dma_start(dst, src)
                # PH[q]/8 gathers of 1024 idx; gather c covers idxr columns
                # [8 PS[q] + 64c, +64) -> qgs rows [PS[q] + 8c, +8).
                for c in range(PH[q] // 8):
                    s_lo = 8 * PS[q] + 64 * c
                    nc.gpsimd.dma_gather(
                        out_ap=qgs[:, s_lo // 8:s_lo // 8 + 8, :],
                        in_ap=w2p[:],
                        idxs_ap=idxr[:, s_lo:s_lo + 64],
                        num_idxs=1024,
                        num_idxs_reg=1024,
                        elem_size=64,
                    )
                nc.sync.dma_start(qg[:, h, :], qgs[:, h, :])

            # ---- Main schedule ----
            for pair in range(NTILE // 2):
                do_pair(pair)
                for q in range(len(PH) - 1):
                    if pair == PS[q + 1] // 2 - 1:
                        extract_quarter(q)
                        gather_quarter(q)
            extract_quarter(len(PH) - 1)
            gather_quarter(len(PH) - 1)
            for c in range(TOK // 512):
                do_ssq(c)
            nc.sync.dma_start(ssqb[:], ssq_sb[:])
            nc.sync.dma_start(mrow[:], mb[:])
            nc.sync.dma_start(idxm[:], idxf[:])

    nc.compile()
    return nc


@functools.lru_cache(maxsize=1)
def _graph():
    return _build_graph()


def _split16(a):
    hi = a.astype(np.float16)
    lo = (a - hi.astype(np.float32)).astype(np.float16)
    return hi, lo


def kernel(z, quant_w, quant_b, codebook, post_quant_w, post_quant_b):
    z = np.asarray(z, dtype=np.float32)
    quant_w = np.asarray(quant_w, dtype=np.float32)
    quant_b = np.asarray(quant_b, dtype=np.float32)
    codebook = np.asarray(codebook, dtype=np.float32)
    post_quant_w = np.asarray(post_quant_w, dtype=np.float32)
    post_quant_b = np.asarray(post_quant_b, dtype=np.float32)

    # ---- Host-side weight folding (tiny, O(K*D)) ----
    cbf = (codebook @ quant_w).astype(np.float32)              # [K, C]
    bias = (codebook @ quant_b
            - 0.5 * np.sum(codebook * codebook, axis=1)).astype(np.float32)
    c0b, c1b = _split16(cbf.T)                                 # [C, K]
    b0, b1 = _split16(bias[None, :])
    c0 = np.concatenate([c0b, b0], axis=0)                     # [33, K] fp16
    c1 = np.concatenate([c1b, b1], axis=0)
    w2 = (codebook @ post_quant_w.T + post_quant_b[None, :]).astype(np.float32)
    w2p = np.zeros((K, 64), np.float32)
    w2p[:, 0:D] = w2
    qwT = quant_w.T.astype(np.float16)                         # [C, D]
    qb = quant_b.reshape(D, 1).astype(np.float32)
    io32 = np.broadcast_to(np.arange(G, dtype=np.float32), (128, G)).copy()
    io16 = np.broadcast_to(np.arange(J, dtype=np.float32), (128, J)).copy()

    in_maps = []
    for core in range(NCORES):
        zc = z[BPC * core:BPC * (core + 1)]                    # [4, C, H, W]
        zc = zc.transpose(1, 0, 2, 3).reshape(C, TOK)          # [C, 16384]
        zh, zl = _split16(zc)
        z0 = np.zeros((34, TOK), np.float16)
        z0[0:C] = zh
        z0[C] = 1.0
        z1 = np.zeros((34, TOK), np.float16)
        z1[0:C] = zl
        in_maps.append({
            "z0": z0, "z1": z1, "c0": c0, "c1": c1, "qwT": qwT, "qb": qb,
            "w2p": w2p, "iota32": io32, "iota16": io16,
        })

    nc = _graph()
    res = bass_utils.run_bass_kernel_spmd(nc, in_maps,
                                          core_ids=list(range(NCORES)))
    outs = res.results

    out = np.empty((B, C, H, W), np.float32)
    idx = np.empty((B, H * W), np.int32)
    ssq_total = 0.0
    smax_total = 0.0
    for core in range(NCORES):
        r = outs[core]
        # qg[16*(tl%8)+p, PS[q] + s8*(PH[q]//8) + tl//8, :] = token
        #   (PS[q] + tl)*128 + 16*s8 + p
        o_tok = np.empty((TOK, 64), np.float32)
        for ph in range(len(PH)):
            n8 = PH[ph] // 8
            qv = r["qg"][:, PS[ph]:PS[ph + 1], :].reshape(8, 16, 8, n8, 64)
            o_tok[PS[ph] * 128:PS[ph + 1] * 128] = np.transpose(
                qv, (3, 0, 2, 1, 4)).reshape(PH[ph] * 128, 64)
        o_tok = o_tok[:, 0:D]
        o_img = o_tok.reshape(BPC, H * W, D).transpose(0, 2, 1)
        out[BPC * core:BPC * (core + 1)] = o_img.reshape(BPC, D, H, W)
        idxc = r["idxm"].T.reshape(TOK).astype(np.int32)
        idx[BPC * core:BPC * (core + 1)] = idxc.reshape(BPC, H * W)
        ssq_total += float(r["ssqb"].sum())
        smax_total += float(r["mrow"].sum())

    n_tok = B * H * W
    loss = np.float32(2.0 * (ssq_total - 2.0 * smax_total) / (n_tok * D))
    return out, idx, loss
